# revision 17
# baseline (speedup 1.0000x reference)
"""AttnBlock (GroupNorm -> q/k/v 1x1 conv -> spatial attention -> proj -> residual)
for B=4, C=512, H=W=64 on 8 TRN2 NeuronCores.

Sharding: core = 2*b + h  (b = batch 0..3, h = query-half 0..1).
Each core group-norms its full batch image, computes k / v^T convs over all
4096 spatial positions, the q conv for its 2048 queries, then flash-style
attention with keys on the partition dim (no max subtraction: scores ~ N(0,1)),
projection and residual for its half. Host reassembles the 8 disjoint halves.
No collectives. Convs run in fp32r (full PE rate), attention in bf16.

Inputs to each core are pre-rotated on host so that its query half is always
columns [0, 2048) -- key order is a permutation, which attention is invariant
to as long as v is permuted identically.
"""

import os
import numpy as np

import concourse.bass as bass
import concourse.tile as tile
from concourse import bacc, mybir
from concourse.bass_utils import run_bass_kernel_spmd

F32 = mybir.dt.float32
F32R = mybir.dt.float32r
BF16 = mybir.dt.bfloat16
AF = mybir.ActivationFunctionType
ALU = mybir.AluOpType

B, C, H, W = 4, 512, 64, 64
HW = H * W            # 4096
HALF = HW // 2        # 2048
G = 32                # groups
GS = C // G           # 16 channels per group
EPS = 1e-6
NKC = C // 128        # 4 channel chunks
NTT = HW // 128       # 32 key tiles
NQB = HALF // 512     # 4 query blocks per half
SCALE = C ** (-0.5)

LAST_EXEC_TIME_NS = None
_CACHED = {}


def _r(x):
    return x.bitcast(F32R)


def _build_program():
    nc = bacc.Bacc("TRN2", target_bir_lowering=False, debug=False)

    feat = nc.dram_tensor("feat", [128, NKC, HW], F32, kind="ExternalInput").ap()
    wq = nc.dram_tensor("wq", [128, NKC, C], F32, kind="ExternalInput").ap()
    wk = nc.dram_tensor("wk", [128, NKC, C], F32, kind="ExternalInput").ap()
    wv = nc.dram_tensor("wv", [128, NKC, C], F32, kind="ExternalInput").ap()
    wp = nc.dram_tensor("wp", [128, NKC, C], F32, kind="ExternalInput").ap()
    bq = nc.dram_tensor("bq", [128, NKC], F32, kind="ExternalInput").ap()
    bk = nc.dram_tensor("bk", [128, NKC], F32, kind="ExternalInput").ap()
    bpe = nc.dram_tensor("bpe", [128, NKC], F32, kind="ExternalInput").ap()
    gw = nc.dram_tensor("gw", [128, NKC], F32, kind="ExternalInput").ap()
    gb = nc.dram_tensor("gb", [128, NKC], F32, kind="ExternalInput").ap()
    sel = nc.dram_tensor("sel", [128, NKC * G], F32, kind="ExternalInput").ap()
    bsel = nc.dram_tensor("bsel", [G, C], F32, kind="ExternalInput").ap()
    out = nc.dram_tensor("out", [128, NKC, HALF], F32, kind="ExternalOutput").ap()

    with tile.TileContext(nc) as tc:
        with (
            tc.tile_pool(name="xn", bufs=NKC) as xn_pool,
            tc.tile_pool(name="wpool", bufs=3) as wpool,
            tc.tile_pool(name="qsb", bufs=1) as q_pool,
            tc.tile_pool(name="ksb", bufs=1) as k_pool,
            tc.tile_pool(name="vsb", bufs=1) as v_pool,
            tc.tile_pool(name="const", bufs=1) as cpool,
            tc.tile_pool(name="stats", bufs=4) as spool,
            tc.tile_pool(name="epool", bufs=4) as epool,
            tc.tile_pool(name="aopool", bufs=4) as aopool,
            tc.tile_pool(name="finpool", bufs=3) as fpool,
            tc.tile_pool(name="rdpool", bufs=2) as rdpool,
            tc.tile_pool(name="stg", bufs=3) as stgpool,
            tc.tile_pool(name="dram", bufs=1, space="DRAM") as dram_pool,
            tc.tile_pool(name="mmps", bufs=2, space="PSUM") as mmps,
            tc.tile_pool(name="scps", bufs=2, space="PSUM") as scps,
            tc.tile_pool(name="avps", bufs=4, space="PSUM") as avps,
            tc.tile_pool(name="dacc", bufs=2) as daccpool,
        ):
            # ---------------- constants ----------------
            sel_sb = cpool.tile([128, NKC * G], F32)
            nc.sync.dma_start(out=sel_sb, in_=sel)
            bsel_sb = cpool.tile([G, C], F32)
            nc.sync.dma_start(out=bsel_sb, in_=bsel)
            bq_sb = cpool.tile([128, NKC], F32)
            nc.sync.dma_start(out=bq_sb, in_=bq)
            bk_sb = cpool.tile([128, NKC], F32)
            nc.sync.dma_start(out=bk_sb, in_=bk)
            bpe_sb = cpool.tile([128, NKC], F32)
            nc.sync.dma_start(out=bpe_sb, in_=bpe)
            gw_sb = cpool.tile([128, NKC], F32)
            nc.sync.dma_start(out=gw_sb, in_=gw)
            gb_sb = cpool.tile([128, NKC], F32)
            nc.sync.dma_start(out=gb_sb, in_=gb)
            ones_sb = cpool.tile([128, 1], F32)
            nc.vector.memset(ones_sb, 1.0)
            ones1_sb = cpool.tile([1, 128], F32)
            nc.vector.memset(ones1_sb, 1.0)
            eps_sb = cpool.tile([G, 1], F32)
            nc.vector.memset(eps_sb, EPS)

            # ---------------- load feature + GN stats ----------------
            f = []
            gsum = mmps.tile([G, 2], F32, tag="mm")
            for kc in range(NKC):
                ft = xn_pool.tile([128, HW], F32, tag="xn")
                for pc in range(8):
                    ps_ = slice(pc * 512, (pc + 1) * 512)
                    nc.sync.dma_start(out=_r(ft[:, ps_]), in_=_r(feat[:, kc, ps_]))
                f.append(ft)
                st = spool.tile([128, 8, 6], F32, tag="bnst")
                for sg in range(8):
                    nc.vector.bn_stats(out=st[:, sg, :], in_=ft[:, sg * 512:(sg + 1) * 512])
                mv = spool.tile([128, 2], F32, tag="mv")
                nc.vector.bn_aggr(out=mv, in_=st)
                # u = [mean_c, E[x^2]_c]
                u = spool.tile([128, 2], F32, tag="u")
                nc.vector.tensor_copy(out=u[:, 0:1], in_=mv[:, 0:1])
                nc.vector.tensor_tensor(out=u[:, 1:2], in0=mv[:, 0:1], in1=mv[:, 0:1], op=ALU.mult)
                nc.vector.tensor_tensor(out=u[:, 1:2], in0=u[:, 1:2], in1=mv[:, 1:2], op=ALU.add)
                nc.tensor.matmul(gsum, lhsT=sel_sb[:, kc * G:(kc + 1) * G], rhs=u,
                                 start=(kc == 0), stop=(kc == NKC - 1))

            # weights: loaded after feat DMAs are queued; wp reuses a slot later
            wq_sb = wpool.tile([128, NKC, C], F32, tag="w")
            nc.sync.dma_start(out=_r(wq_sb), in_=_r(wq))
            wk_sb = wpool.tile([128, NKC, C], F32, tag="w")
            nc.sync.dma_start(out=_r(wk_sb), in_=_r(wk))
            wv_sb = wpool.tile([128, NKC, C], F32, tag="w")
            nc.sync.dma_start(out=_r(wv_sb), in_=_r(wv))

            # group stats -> per-group [mean_g, rstd_g]
            gsb = spool.tile([G, 2], F32, tag="gsb")
            nc.vector.tensor_copy(out=gsb, in_=gsum)
            gm2 = spool.tile([G, 1], F32, tag="gtmp")
            nc.vector.tensor_tensor(out=gm2, in0=gsb[:, 0:1], in1=gsb[:, 0:1], op=ALU.mult)
            gv = spool.tile([G, 1], F32, tag="gtmp2")
            nc.vector.tensor_tensor(out=gv, in0=gsb[:, 1:2], in1=gm2, op=ALU.subtract)
            gvals = spool.tile([G, 2], F32, tag="gvals")
            gsd = spool.tile([G, 1], F32, tag="gsd")
            nc.scalar.activation(out=gsd, in_=gv, func=AF.Sqrt, bias=eps_sb, scale=1.0)
            nc.vector.reciprocal(out=gvals[:, 1:2], in_=gsd)
            nc.vector.tensor_copy(out=gvals[:, 0:1], in_=gsb[:, 0:1])

            # broadcast to per-channel affine, normalize in place
            for kc in range(NKC):
                cm = mmps.tile([128, 2], F32, tag="mm")
                nc.tensor.matmul(cm, lhsT=bsel_sb[:, kc * 128:(kc + 1) * 128], rhs=gvals,
                                 start=True, stop=True)
                a = spool.tile([128, 1], F32, tag="aff_a")
                nc.vector.tensor_tensor(out=a, in0=cm[:, 1:2], in1=gw_sb[:, kc:kc + 1], op=ALU.mult)
                bb = spool.tile([128, 1], F32, tag="aff_b")
                nc.vector.tensor_tensor(out=bb, in0=cm[:, 0:1], in1=a, op=ALU.mult)
                nc.vector.tensor_tensor(out=bb, in0=gb_sb[:, kc:kc + 1], in1=bb, op=ALU.subtract)
                for pc in range(4):
                    cs = slice(pc * 1024, (pc + 1) * 1024)
                    nc.vector.tensor_scalar(out=_r(f[kc][:, cs]), in0=f[kc][:, cs],
                                            scalar1=a, scalar2=bb,
                                            op0=ALU.mult, op1=ALU.add)

            # ---------------- q / k / vT convs (fp32r) ----------------
            # Each core computes k and vT only for its LOCAL half of keys
            # (local cols [0, HALF), which are original keys [h*HALF, ...)),
            # then pair-AllGathers them into canonical key order. Key order
            # in attention is a free permutation as long as k and vT agree.
            RG = [[0, 1], [2, 3], [4, 5], [6, 7]]

            # k conv: local half -> stage -> DRAM bounce -> AllGather
            kag_in = dram_pool.tile([128, NKC, HALF], BF16)
            kag_out = dram_pool.tile([2, 128, NKC, HALF], BF16)
            for nb in range(HALF // 512):
                for mo in range(NKC):
                    ps = mmps.tile([128, 512], F32, tag="mm")
                    for kc in range(NKC):
                        nc.tensor.matmul(ps, lhsT=_r(wk_sb[:, kc, mo * 128:(mo + 1) * 128]),
                                         rhs=_r(f[kc][:, nb * 512:(nb + 1) * 512]),
                                         start=(kc == 0), stop=(kc == NKC - 1))
                    stg = stgpool.tile([128, 512], BF16, tag="stg")
                    nc.vector.tensor_scalar(out=stg, in0=ps,
                                            scalar1=bk_sb[:, mo:mo + 1], scalar2=None,
                                            op0=ALU.add)
                    nc.sync.dma_start(out=kag_in[:, mo, nb * 512:(nb + 1) * 512], in_=stg)
            nc.gpsimd.collective_compute(
                "AllGather", ALU.bypass, replica_groups=RG,
                ins=[kag_in.opt()], outs=[kag_out.opt()])

            # q conv (runs on PE while the k AllGather is in flight)
            q_sb = q_pool.tile([128, NKC, HALF], BF16)
            for mo in range(NKC):
                for qb in range(NQB):
                    ps = mmps.tile([128, 512], F32, tag="mm")
                    for kc in range(NKC):
                        nc.tensor.matmul(ps, lhsT=_r(wq_sb[:, kc, mo * 128:(mo + 1) * 128]),
                                         rhs=_r(f[kc][:, qb * 512:(qb + 1) * 512]),
                                         start=(kc == 0), stop=(kc == NKC - 1))
                    nc.vector.tensor_scalar(out=q_sb[:, mo, qb * 512:(qb + 1) * 512], in0=ps,
                                            scalar1=bq_sb[:, mo:mo + 1], scalar2=None,
                                            op0=ALU.add)

            # vT conv: local 16 key tiles -> stage -> bounce -> AllGather
            vag_in = dram_pool.tile([128, NTT // 2, C], BF16)
            vag_out = dram_pool.tile([2, 128, NTT // 2, C], BF16)
            for tt in range(NTT // 2):
                ps = mmps.tile([128, 512], F32, tag="mm")
                for kc in range(NKC):
                    nc.tensor.matmul(ps, lhsT=_r(f[kc][:, tt * 128:(tt + 1) * 128]),
                                     rhs=_r(wv_sb[:, kc, :]),
                                     start=(kc == 0), stop=(kc == NKC - 1))
                stg = stgpool.tile([128, 512], BF16, tag="stg")
                nc.vector.tensor_copy(out=stg, in_=ps)
                nc.sync.dma_start(out=vag_in[:, tt, :], in_=stg)
            nc.gpsimd.collective_compute(
                "AllGather", ALU.bypass, replica_groups=RG,
                ins=[vag_in.opt()], outs=[vag_out.opt()])

            # proj weights into slot freed by wq
            wp_sb = wpool.tile([128, NKC, C], F32, tag="w")
            nc.sync.dma_start(out=_r(wp_sb), in_=_r(wp))

            # reload gathered k / vT into SBUF in canonical key order
            k_sb = k_pool.tile([128, NKC, HW], BF16)
            nc.sync.dma_start(out=k_sb[:, :, 0:HALF], in_=kag_out[0])
            nc.sync.dma_start(out=k_sb[:, :, HALF:HW], in_=kag_out[1])
            vT_sb = v_pool.tile([128, NTT, C], BF16)
            nc.sync.dma_start(out=vT_sb[:, 0:NTT // 2, :], in_=vag_out[0])
            nc.sync.dma_start(out=vT_sb[:, NTT // 2:NTT, :], in_=vag_out[1])

            # ---------------- attention per query block ----------------
            for qb in range(NQB):
                qs = slice(qb * 512, (qb + 1) * 512)
                av = [avps.tile([128, 512], F32, tag="av", name=f"av{qb}_{i}") for i in range(NKC)]
                acc = daccpool.tile([128, 512], F32, tag="dacc", name=f"dacc{qb}")

                def emit_scores(tt):
                    sc = scps.tile([128, 512], F32, tag="sc", name=f"sc{qb}_{tt}")
                    for kc in range(NKC):
                        nc.tensor.matmul(sc, lhsT=k_sb[:, kc, tt * 128:(tt + 1) * 128],
                                         rhs=q_sb[:, kc, qs],
                                         start=(kc == 0), stop=(kc == NKC - 1))
                    return sc

                # software pipeline: PE runs scores[tt+1] while ACT does exp[tt]
                sc_prev = emit_scores(0)
                for tt in range(NTT):
                    e = epool.tile([128, 512], BF16, tag="e")
                    nc.scalar.activation(out=e, in_=sc_prev, func=AF.Exp)
                    if tt + 1 < NTT:
                        sc_prev = emit_scores(tt + 1)
                    if tt == 0:
                        nc.vector.tensor_copy(out=acc, in_=e)
                    else:
                        nc.vector.tensor_tensor(out=acc, in0=acc, in1=e, op=ALU.add)
                    for mo in range(NKC):
                        nc.tensor.matmul(av[mo], lhsT=vT_sb[:, tt, mo * 128:(mo + 1) * 128],
                                         rhs=e,
                                         start=(tt == 0), stop=(tt == NTT - 1),
                                         skip_group_check=True)

                # denominator: partition-sum of acc, reciprocal, broadcast
                den_ps = mmps.tile([1, 512], F32, tag="mm", name=f"den{qb}")
                nc.tensor.matmul(den_ps, lhsT=ones_sb, rhs=acc, start=True, stop=True)
                rden = rdpool.tile([1, 512], F32, tag="rden")
                nc.vector.reciprocal(out=rden, in_=den_ps)
                rden_ps = mmps.tile([128, 512], F32, tag="mm", name=f"rdps{qb}")
                nc.tensor.matmul(rden_ps, lhsT=ones1_sb, rhs=rden, start=True, stop=True)
                rden_b = rdpool.tile([128, 512], F32, tag="rdenb")
                nc.vector.tensor_copy(out=rden_b, in_=rden_ps)

                # unnormalized attention out -> SBUF (frees av banks fast);
                # normalization commutes with the (linear) projection
                ao = []
                for mo in range(NKC):
                    t = aopool.tile([128, 512], F32, tag="ao", name=f"ao{qb}_{mo}")
                    nc.vector.tensor_copy(out=_r(t), in_=av[mo])
                    ao.append(t)

                for mo in range(NKC):
                    pp = mmps.tile([128, 512], F32, tag="mm")
                    for kc in range(NKC):
                        nc.tensor.matmul(pp, lhsT=_r(wp_sb[:, kc, mo * 128:(mo + 1) * 128]),
                                         rhs=_r(ao[kc]),
                                         start=(kc == 0), stop=(kc == NKC - 1))
                    res = fpool.tile([128, 512], F32, tag="fin")
                    nc.sync.dma_start(out=res, in_=feat[:, mo, qs])
                    t0 = fpool.tile([128, 512], F32, tag="fin")
                    nc.vector.tensor_scalar(out=t0, in0=res,
                                            scalar1=bpe_sb[:, mo:mo + 1], scalar2=None,
                                            op0=ALU.add)
                    f1 = fpool.tile([128, 512], F32, tag="fin")
                    nc.vector.tensor_tensor(out=f1, in0=pp, in1=rden_b, op=ALU.mult)
                    fin = fpool.tile([128, 512], F32, tag="fin")
                    nc.vector.tensor_tensor(out=fin, in0=f1, in1=t0, op=ALU.add)
                    nc.sync.dma_start(out=out[:, mo, qs], in_=fin)

    nc.compile()
    return nc


def _chunk_cols(a):
    # (C,) -> (128, NKC) with [p, kc] = a[kc*128+p]
    return np.ascontiguousarray(a.reshape(NKC, 128).T)


def _chunk_wT(w, scale=1.0):
    # (O, Cin) -> lhsT chunks (128, NKC, O): [p, kc, o] = w[o, kc*128+p]*scale
    return np.ascontiguousarray((w.T * scale).reshape(NKC, 128, C).transpose(1, 0, 2))


def kernel(feature, gn_gamma, gn_beta, wq, bq, wk, bk, wv, bv, wp, bp):
    global LAST_EXEC_TIME_NS
    feature = np.asarray(feature, np.float32)
    wq, bq = np.asarray(wq, np.float32), np.asarray(bq, np.float32)
    wk, bk = np.asarray(wk, np.float32), np.asarray(bk, np.float32)
    wv, bv = np.asarray(wv, np.float32), np.asarray(bv, np.float32)
    wp, bp = np.asarray(wp, np.float32), np.asarray(bp, np.float32)
    gn_gamma, gn_beta = np.asarray(gn_gamma, np.float32), np.asarray(gn_beta, np.float32)

    if "nc" not in _CACHED:
        _CACHED["nc"] = _build_program()
    nc = _CACHED["nc"]

    sel = np.zeros((128, NKC * G), np.float32)
    bsel = np.zeros((G, C), np.float32)
    for kc in range(NKC):
        for p in range(128):
            g = 8 * kc + p // GS
            sel[p, kc * G + g] = 1.0 / GS
            bsel[g, kc * 128 + p] = 1.0

    bpe = wp @ bv + bp
    shared = {
        "wq": _chunk_wT(wq, SCALE), "wk": _chunk_wT(wk), "wv": _chunk_wT(wv),
        "wp": _chunk_wT(wp),
        "bq": _chunk_cols(bq * SCALE), "bk": _chunk_cols(bk), "bpe": _chunk_cols(bpe),
        "gw": _chunk_cols(gn_gamma), "gb": _chunk_cols(gn_beta),
        "sel": sel, "bsel": bsel,
    }

    fx = feature.reshape(B, C, HW)
    in_maps = []
    for core in range(8):
        b, h = core // 2, core % 2
        fb = fx[b]
        if h:
            fb = np.concatenate([fb[:, HALF:], fb[:, :HALF]], axis=1)
        fb = np.ascontiguousarray(fb.reshape(NKC, 128, HW).transpose(1, 0, 2))
        in_maps.append({"feat": fb, **shared})

    trace = bool(int(os.environ.get("BASS_KERNEL_TRACE", "0")))
    try:
        r = run_bass_kernel_spmd(nc, in_maps, list(range(8)), trace=trace)
    except (ImportError, ModuleNotFoundError):
        r = run_bass_kernel_spmd(nc, in_maps, list(range(8)), trace=False)
    LAST_EXEC_TIME_NS = r.exec_time_ns

    outf = np.empty((B, C, HW), np.float32)
    for core in range(8):
        b, h = core // 2, core % 2
        o = r.results[core]["out"]  # (128, NKC, HALF)
        outf[b][:, h * HALF:(h + 1) * HALF] = o.transpose(1, 0, 2).reshape(C, HALF)
    return outf.reshape(B, C, H, W)


# revision 21
# speedup vs baseline: 1.0395x; 1.0395x over previous
"""AttnBlock (GroupNorm -> q/k/v 1x1 conv -> spatial attention -> proj -> residual)
for B=4, C=512, H=W=64 on 8 TRN2 NeuronCores.

Sharding: core = 2*b + h  (b = batch 0..3, h = query-half 0..1).
Each core group-norms its full batch image, computes k / v^T convs over all
4096 spatial positions, the q conv for its 2048 queries, then flash-style
attention with keys on the partition dim (no max subtraction: scores ~ N(0,1)),
projection and residual for its half. Host reassembles the 8 disjoint halves.
No collectives. Convs run in fp32r (full PE rate), attention in bf16.

Inputs to each core are pre-rotated on host so that its query half is always
columns [0, 2048) -- key order is a permutation, which attention is invariant
to as long as v is permuted identically.
"""

import os
import numpy as np

import concourse.bass as bass
import concourse.tile as tile
from concourse import bacc, mybir
from concourse.bass_utils import run_bass_kernel_spmd

F32 = mybir.dt.float32
F32R = mybir.dt.float32r
BF16 = mybir.dt.bfloat16
AF = mybir.ActivationFunctionType
ALU = mybir.AluOpType

B, C, H, W = 4, 512, 64, 64
HW = H * W            # 4096
HALF = HW // 2        # 2048
G = 32                # groups
GS = C // G           # 16 channels per group
EPS = 1e-6
NKC = C // 128        # 4 channel chunks
NTT = HW // 128       # 32 key tiles
NQB = HALF // 512     # 4 query blocks per half
SCALE = C ** (-0.5)

LAST_EXEC_TIME_NS = None
_CACHED = {}


def _r(x):
    return x.bitcast(F32R)


def _build_program():
    nc = bacc.Bacc("TRN2", target_bir_lowering=False, debug=False)

    feat = nc.dram_tensor("feat", [128, NKC, HW], F32, kind="ExternalInput").ap()
    wq = nc.dram_tensor("wq", [128, NKC, C], F32, kind="ExternalInput").ap()
    wk = nc.dram_tensor("wk", [128, NKC, C], F32, kind="ExternalInput").ap()
    wv = nc.dram_tensor("wv", [128, NKC, C], F32, kind="ExternalInput").ap()
    wp = nc.dram_tensor("wp", [128, NKC, C], F32, kind="ExternalInput").ap()
    bq = nc.dram_tensor("bq", [128, NKC], F32, kind="ExternalInput").ap()
    bk = nc.dram_tensor("bk", [128, NKC], F32, kind="ExternalInput").ap()
    bpe = nc.dram_tensor("bpe", [128, NKC], F32, kind="ExternalInput").ap()
    gw = nc.dram_tensor("gw", [128, NKC], F32, kind="ExternalInput").ap()
    gb = nc.dram_tensor("gb", [128, NKC], F32, kind="ExternalInput").ap()
    sel = nc.dram_tensor("sel", [128, NKC * G], F32, kind="ExternalInput").ap()
    bsel = nc.dram_tensor("bsel", [G, C], F32, kind="ExternalInput").ap()
    out = nc.dram_tensor("out", [128, NKC, HALF], F32, kind="ExternalOutput").ap()

    with tile.TileContext(nc) as tc:
        with (
            tc.tile_pool(name="xn", bufs=NKC) as xn_pool,
            tc.tile_pool(name="wpool", bufs=3) as wpool,
            tc.tile_pool(name="qsb", bufs=1) as q_pool,
            tc.tile_pool(name="ksb", bufs=1) as k_pool,
            tc.tile_pool(name="vsb", bufs=1) as v_pool,
            tc.tile_pool(name="const", bufs=1) as cpool,
            tc.tile_pool(name="stats", bufs=4) as spool,
            tc.tile_pool(name="epool", bufs=4) as epool,
            tc.tile_pool(name="aopool", bufs=4) as aopool,
            tc.tile_pool(name="finpool", bufs=3) as fpool,
            tc.tile_pool(name="rdpool", bufs=2) as rdpool,
            tc.tile_pool(name="stg", bufs=3) as stgpool,
            tc.tile_pool(name="dram", bufs=1, space="DRAM") as dram_pool,
            tc.tile_pool(name="mmps", bufs=2, space="PSUM") as mmps,
            tc.tile_pool(name="scps", bufs=2, space="PSUM") as scps,
            tc.tile_pool(name="avps", bufs=4, space="PSUM") as avps,
            tc.tile_pool(name="dacc", bufs=2) as daccpool,
        ):
            # ---------------- constants ----------------
            sel_sb = cpool.tile([128, NKC * G], F32)
            nc.sync.dma_start(out=sel_sb, in_=sel)
            bsel_sb = cpool.tile([G, C], F32)
            nc.sync.dma_start(out=bsel_sb, in_=bsel)
            bq_sb = cpool.tile([128, NKC], F32)
            nc.sync.dma_start(out=bq_sb, in_=bq)
            bk_sb = cpool.tile([128, NKC], F32)
            nc.sync.dma_start(out=bk_sb, in_=bk)
            bpe_sb = cpool.tile([128, NKC], F32)
            nc.sync.dma_start(out=bpe_sb, in_=bpe)
            gw_sb = cpool.tile([128, NKC], F32)
            nc.sync.dma_start(out=gw_sb, in_=gw)
            gb_sb = cpool.tile([128, NKC], F32)
            nc.sync.dma_start(out=gb_sb, in_=gb)
            ones_sb = cpool.tile([128, 1], F32)
            nc.vector.memset(ones_sb, 1.0)
            ones1_sb = cpool.tile([1, 128], F32)
            nc.vector.memset(ones1_sb, 1.0)
            eps_sb = cpool.tile([G, 1], F32)
            nc.vector.memset(eps_sb, EPS)

            # ---------------- load feature + GN stats ----------------
            f = []
            sts = []
            gsum = mmps.tile([G, 2], F32, tag="mm")
            for kc in range(NKC):
                ft = xn_pool.tile([128, HW], F32, tag="xn", name=f"ft{kc}")
                f.append(ft)
                st = spool.tile([128, 8, 6], F32, tag="bnst", name=f"st{kc}")
                sts.append(st)
            # round-robin pieces across chunks: the last bn_stats completes
            # right after the last DMA piece lands
            for pc in range(8):
                ps_ = slice(pc * 512, (pc + 1) * 512)
                for kc in range(NKC):
                    nc.sync.dma_start(out=_r(f[kc][:, ps_]), in_=_r(feat[:, kc, ps_]))
            for pc in range(8):
                ps_ = slice(pc * 512, (pc + 1) * 512)
                for kc in range(NKC):
                    nc.vector.bn_stats(out=sts[kc][:, pc, :], in_=f[kc][:, ps_])
            for kc in range(NKC):
                mv = spool.tile([128, 2], F32, tag="mv")
                nc.vector.bn_aggr(out=mv, in_=sts[kc])
                # u = [mean_c, E[x^2]_c]
                u = spool.tile([128, 2], F32, tag="u")
                nc.vector.tensor_copy(out=u[:, 0:1], in_=mv[:, 0:1])
                nc.vector.tensor_tensor(out=u[:, 1:2], in0=mv[:, 0:1], in1=mv[:, 0:1], op=ALU.mult)
                nc.vector.tensor_tensor(out=u[:, 1:2], in0=u[:, 1:2], in1=mv[:, 1:2], op=ALU.add)
                nc.tensor.matmul(gsum, lhsT=sel_sb[:, kc * G:(kc + 1) * G], rhs=u,
                                 start=(kc == 0), stop=(kc == NKC - 1))

            # weights: loaded after feat DMAs are queued; wp reuses a slot later
            wk_sb = wpool.tile([128, NKC, C], F32, tag="w")
            nc.sync.dma_start(out=_r(wk_sb), in_=_r(wk))
            wq_sb = wpool.tile([128, NKC, C], F32, tag="w")
            nc.sync.dma_start(out=_r(wq_sb), in_=_r(wq))
            wv_sb = wpool.tile([128, NKC, C], F32, tag="w")
            nc.sync.dma_start(out=_r(wv_sb), in_=_r(wv))

            # group stats -> per-group [mean_g, rstd_g]
            gsb = spool.tile([G, 2], F32, tag="gsb")
            nc.vector.tensor_copy(out=gsb, in_=gsum)
            gm2 = spool.tile([G, 1], F32, tag="gtmp")
            nc.vector.tensor_tensor(out=gm2, in0=gsb[:, 0:1], in1=gsb[:, 0:1], op=ALU.mult)
            gv = spool.tile([G, 1], F32, tag="gtmp2")
            nc.vector.tensor_tensor(out=gv, in0=gsb[:, 1:2], in1=gm2, op=ALU.subtract)
            gvals = spool.tile([G, 2], F32, tag="gvals")
            gsd = spool.tile([G, 1], F32, tag="gsd")
            nc.scalar.activation(out=gsd, in_=gv, func=AF.Sqrt, bias=eps_sb, scale=1.0)
            nc.vector.reciprocal(out=gvals[:, 1:2], in_=gsd)
            nc.vector.tensor_copy(out=gvals[:, 0:1], in_=gsb[:, 0:1])

            # broadcast to per-channel affine, normalize in place
            for kc in range(NKC):
                cm = mmps.tile([128, 2], F32, tag="mm")
                nc.tensor.matmul(cm, lhsT=bsel_sb[:, kc * 128:(kc + 1) * 128], rhs=gvals,
                                 start=True, stop=True)
                a = spool.tile([128, 1], F32, tag="aff_a")
                nc.vector.tensor_tensor(out=a, in0=cm[:, 1:2], in1=gw_sb[:, kc:kc + 1], op=ALU.mult)
                bb = spool.tile([128, 1], F32, tag="aff_b")
                nc.vector.tensor_tensor(out=bb, in0=cm[:, 0:1], in1=a, op=ALU.mult)
                nc.vector.tensor_tensor(out=bb, in0=gb_sb[:, kc:kc + 1], in1=bb, op=ALU.subtract)
                for pc in range(4):
                    cs = slice(pc * 1024, (pc + 1) * 1024)
                    nc.vector.tensor_scalar(out=_r(f[kc][:, cs]), in0=f[kc][:, cs],
                                            scalar1=a, scalar2=bb,
                                            op0=ALU.mult, op1=ALU.add)

            # ---------------- q / k / vT convs (fp32r) ----------------
            # Each core computes k and vT only for its LOCAL half of keys
            # (local cols [0, HALF), which are original keys [h*HALF, ...)),
            # then pair-AllGathers them into canonical key order. Key order
            # in attention is a free permutation as long as k and vT agree.
            RG = [[0, 1], [2, 3], [4, 5], [6, 7]]

            # k conv: local half -> stage -> DRAM bounce -> AllGather
            kag_in = dram_pool.tile([128, NKC, HALF], BF16)
            kag_out = dram_pool.tile([2, 128, NKC, HALF], BF16)
            for nb in range(HALF // 512):
                for mo in range(NKC):
                    ps = mmps.tile([128, 512], F32, tag="mm")
                    for kc in range(NKC):
                        nc.tensor.matmul(ps, lhsT=_r(wk_sb[:, kc, mo * 128:(mo + 1) * 128]),
                                         rhs=_r(f[kc][:, nb * 512:(nb + 1) * 512]),
                                         start=(kc == 0), stop=(kc == NKC - 1))
                    stg = stgpool.tile([128, 512], BF16, tag="stg")
                    nc.vector.tensor_scalar(out=stg, in0=ps,
                                            scalar1=bk_sb[:, mo:mo + 1], scalar2=None,
                                            op0=ALU.add)
                    nc.sync.dma_start(out=kag_in[:, mo, nb * 512:(nb + 1) * 512], in_=stg)
            nc.gpsimd.collective_compute(
                "AllGather", ALU.bypass, replica_groups=RG,
                ins=[kag_in.opt()], outs=[kag_out.opt()])

            # q conv (runs on PE while the k AllGather is in flight)
            q_sb = q_pool.tile([128, NKC, HALF], BF16)
            for mo in range(NKC):
                for qb in range(NQB):
                    ps = mmps.tile([128, 512], F32, tag="mm")
                    for kc in range(NKC):
                        nc.tensor.matmul(ps, lhsT=_r(wq_sb[:, kc, mo * 128:(mo + 1) * 128]),
                                         rhs=_r(f[kc][:, qb * 512:(qb + 1) * 512]),
                                         start=(kc == 0), stop=(kc == NKC - 1))
                    nc.vector.tensor_scalar(out=q_sb[:, mo, qb * 512:(qb + 1) * 512], in0=ps,
                                            scalar1=bq_sb[:, mo:mo + 1], scalar2=None,
                                            op0=ALU.add)

            # vT conv: local 16 key tiles -> stage -> bounce -> AllGather
            vag_in = dram_pool.tile([128, NTT // 2, C], BF16)
            vag_out = dram_pool.tile([2, 128, NTT // 2, C], BF16)
            for tt in range(NTT // 2):
                ps = mmps.tile([128, 512], F32, tag="mm")
                for kc in range(NKC):
                    nc.tensor.matmul(ps, lhsT=_r(f[kc][:, tt * 128:(tt + 1) * 128]),
                                     rhs=_r(wv_sb[:, kc, :]),
                                     start=(kc == 0), stop=(kc == NKC - 1))
                stg = stgpool.tile([128, 512], BF16, tag="stg")
                nc.vector.tensor_copy(out=stg, in_=ps)
                nc.sync.dma_start(out=vag_in[:, tt, :], in_=stg)
            nc.gpsimd.collective_compute(
                "AllGather", ALU.bypass, replica_groups=RG,
                ins=[vag_in.opt()], outs=[vag_out.opt()])

            # proj weights into slot freed by wq
            wp_sb = wpool.tile([128, NKC, C], F32, tag="w")
            nc.sync.dma_start(out=_r(wp_sb), in_=_r(wp))

            # reload gathered k / vT into SBUF in canonical key order
            k_sb = k_pool.tile([128, NKC, HW], BF16)
            nc.sync.dma_start(out=k_sb[:, :, 0:HALF], in_=kag_out[0])
            nc.sync.dma_start(out=k_sb[:, :, HALF:HW], in_=kag_out[1])
            vT_sb = v_pool.tile([128, NTT, C], BF16)
            nc.sync.dma_start(out=vT_sb[:, 0:NTT // 2, :], in_=vag_out[0])
            nc.sync.dma_start(out=vT_sb[:, NTT // 2:NTT, :], in_=vag_out[1])

            # ---------------- attention per query block ----------------
            for qb in range(NQB):
                qs = slice(qb * 512, (qb + 1) * 512)
                av = [avps.tile([128, 512], F32, tag="av", name=f"av{qb}_{i}") for i in range(NKC)]
                acc = daccpool.tile([128, 512], F32, tag="dacc", name=f"dacc{qb}")

                def emit_scores(tt):
                    sc = scps.tile([128, 512], F32, tag="sc", name=f"sc{qb}_{tt}")
                    for kc in range(NKC):
                        nc.tensor.matmul(sc, lhsT=k_sb[:, kc, tt * 128:(tt + 1) * 128],
                                         rhs=q_sb[:, kc, qs],
                                         start=(kc == 0), stop=(kc == NKC - 1))
                    return sc

                # software pipeline: PE runs scores[tt+1] while ACT does exp[tt]
                sc_prev = emit_scores(0)
                for tt in range(NTT):
                    e = epool.tile([128, 512], BF16, tag="e")
                    nc.scalar.activation(out=e, in_=sc_prev, func=AF.Exp)
                    if tt + 1 < NTT:
                        sc_prev = emit_scores(tt + 1)
                    if tt == 0:
                        nc.vector.tensor_copy(out=acc, in_=e)
                    else:
                        nc.vector.tensor_tensor(out=acc, in0=acc, in1=e, op=ALU.add)
                    for mo in range(NKC):
                        nc.tensor.matmul(av[mo], lhsT=vT_sb[:, tt, mo * 128:(mo + 1) * 128],
                                         rhs=e,
                                         start=(tt == 0), stop=(tt == NTT - 1),
                                         skip_group_check=True)

                # denominator: partition-sum of acc, reciprocal, broadcast
                den_ps = mmps.tile([1, 512], F32, tag="mm", name=f"den{qb}")
                nc.tensor.matmul(den_ps, lhsT=ones_sb, rhs=acc, start=True, stop=True)
                rden = rdpool.tile([1, 512], F32, tag="rden")
                nc.vector.reciprocal(out=rden, in_=den_ps)
                rden_ps = mmps.tile([128, 512], F32, tag="mm", name=f"rdps{qb}")
                nc.tensor.matmul(rden_ps, lhsT=ones1_sb, rhs=rden, start=True, stop=True)
                rden_b = rdpool.tile([128, 512], F32, tag="rdenb")
                nc.vector.tensor_copy(out=rden_b, in_=rden_ps)

                # unnormalized attention out -> SBUF (frees av banks fast);
                # normalization commutes with the (linear) projection
                ao = []
                for mo in range(NKC):
                    t = aopool.tile([128, 512], F32, tag="ao", name=f"ao{qb}_{mo}")
                    nc.vector.tensor_copy(out=_r(t), in_=av[mo])
                    ao.append(t)

                for mo in range(NKC):
                    pp = mmps.tile([128, 512], F32, tag="mm")
                    for kc in range(NKC):
                        nc.tensor.matmul(pp, lhsT=_r(wp_sb[:, kc, mo * 128:(mo + 1) * 128]),
                                         rhs=_r(ao[kc]),
                                         start=(kc == 0), stop=(kc == NKC - 1))
                    res = fpool.tile([128, 512], F32, tag="fin")
                    nc.sync.dma_start(out=res, in_=feat[:, mo, qs])
                    t0 = fpool.tile([128, 512], F32, tag="fin")
                    nc.vector.tensor_scalar(out=t0, in0=res,
                                            scalar1=bpe_sb[:, mo:mo + 1], scalar2=None,
                                            op0=ALU.add)
                    f1 = fpool.tile([128, 512], F32, tag="fin")
                    nc.vector.tensor_tensor(out=f1, in0=pp, in1=rden_b, op=ALU.mult)
                    fin = fpool.tile([128, 512], F32, tag="fin")
                    nc.vector.tensor_tensor(out=fin, in0=f1, in1=t0, op=ALU.add)
                    nc.sync.dma_start(out=out[:, mo, qs], in_=fin)

    nc.compile()
    return nc


def _chunk_cols(a):
    # (C,) -> (128, NKC) with [p, kc] = a[kc*128+p]
    return np.ascontiguousarray(a.reshape(NKC, 128).T)


def _chunk_wT(w, scale=1.0):
    # (O, Cin) -> lhsT chunks (128, NKC, O): [p, kc, o] = w[o, kc*128+p]*scale
    return np.ascontiguousarray((w.T * scale).reshape(NKC, 128, C).transpose(1, 0, 2))


def kernel(feature, gn_gamma, gn_beta, wq, bq, wk, bk, wv, bv, wp, bp):
    global LAST_EXEC_TIME_NS
    feature = np.asarray(feature, np.float32)
    wq, bq = np.asarray(wq, np.float32), np.asarray(bq, np.float32)
    wk, bk = np.asarray(wk, np.float32), np.asarray(bk, np.float32)
    wv, bv = np.asarray(wv, np.float32), np.asarray(bv, np.float32)
    wp, bp = np.asarray(wp, np.float32), np.asarray(bp, np.float32)
    gn_gamma, gn_beta = np.asarray(gn_gamma, np.float32), np.asarray(gn_beta, np.float32)

    if "nc" not in _CACHED:
        _CACHED["nc"] = _build_program()
    nc = _CACHED["nc"]

    sel = np.zeros((128, NKC * G), np.float32)
    bsel = np.zeros((G, C), np.float32)
    for kc in range(NKC):
        for p in range(128):
            g = 8 * kc + p // GS
            sel[p, kc * G + g] = 1.0 / GS
            bsel[g, kc * 128 + p] = 1.0

    bpe = wp @ bv + bp
    shared = {
        "wq": _chunk_wT(wq, SCALE), "wk": _chunk_wT(wk), "wv": _chunk_wT(wv),
        "wp": _chunk_wT(wp),
        "bq": _chunk_cols(bq * SCALE), "bk": _chunk_cols(bk), "bpe": _chunk_cols(bpe),
        "gw": _chunk_cols(gn_gamma), "gb": _chunk_cols(gn_beta),
        "sel": sel, "bsel": bsel,
    }

    fx = feature.reshape(B, C, HW)
    in_maps = []
    for core in range(8):
        b, h = core // 2, core % 2
        fb = fx[b]
        if h:
            fb = np.concatenate([fb[:, HALF:], fb[:, :HALF]], axis=1)
        fb = np.ascontiguousarray(fb.reshape(NKC, 128, HW).transpose(1, 0, 2))
        in_maps.append({"feat": fb, **shared})

    trace = bool(int(os.environ.get("BASS_KERNEL_TRACE", "0")))
    try:
        r = run_bass_kernel_spmd(nc, in_maps, list(range(8)), trace=trace)
    except (ImportError, ModuleNotFoundError):
        r = run_bass_kernel_spmd(nc, in_maps, list(range(8)), trace=False)
    LAST_EXEC_TIME_NS = r.exec_time_ns

    outf = np.empty((B, C, HW), np.float32)
    for core in range(8):
        b, h = core // 2, core % 2
        o = r.results[core]["out"]  # (128, NKC, HALF)
        outf[b][:, h * HALF:(h + 1) * HALF] = o.transpose(1, 0, 2).reshape(C, HALF)
    return outf.reshape(B, C, H, W)
